# revision 35
# baseline (speedup 1.0000x reference)
"""Trainium2 Bass kernel for nn_Attention (B=16,N=4096,C=1024,H=16,HD=64,Q=64).

Data-parallel over B across 8 NeuronCores (2 batches/core). Per batch the
attention is reassociated so no k/v tensors are materialized and no on-chip
transposes are needed:

  q^T = Wq @ x_q^T                      [(h,d)=1024, 64]
  G_h^T = Wk_h^T @ q_h                  G^T: [c=1024, (h,q)=1024]
  S^T   = x @ G^T   (per t-tile)        [t, (h,q)]   (contract c)
  p^T   = exp(S^T / 8)                  (softmax w/o max-sub: scores ~ +-5)
  u^T   = x^T(nat) @ p^T  (accum t)     [c, (h,q)]   (contract t)
  den   = ones @ pacc     (pacc: GpSimd p-sum over t)
  o_h^T = (Wv_h^T)^T @ u_h^T, scaled by 1/den at PSUM eviction
  y     = o^T.T @ Wproj^T + b           [64, 1024]   (contract (h,d))

Both batches run as one flat 16-block pipeline: u-rounds are delayed one
block behind the S/exp stream so the scalar-engine exp always has a full
block of slack, and the q/G prologue + o/proj epilogue batch the two
batches through shared stationary weights (moving dim 128/256 instead of
64).  Epilogue-only weights (Wv/Wproj) are DMAed mid-t-loop so the
prologue weights get full HBM bandwidth.  The u accumulator is bf16 in
SBUF (both batches in one tile) so o's moving operand can span batches
with a strided AP.
"""
import numpy as np

B, N, C = 16, 4096, 1024
H, HD, QL = 16, 64, 64
BL = B // 8           # batches per core
CK = C // 128         # 8 c-tiles
TB = 512              # tokens per t-block
NBLK = N // TB        # 8 blocks per batch
TPB = TB // 128       # 4 t-tiles per block
VB = BL * NBLK        # 16 virtual blocks
HQ = H * QL           # 1024
SCALE = HD ** -0.5

_CACHE = {}


def _build():
    import concourse.bass as bass
    import concourse.tile as tile
    from concourse import bacc, bass_isa, mybir

    f32 = mybir.dt.float32
    bf16 = mybir.dt.bfloat16
    EXP = mybir.ActivationFunctionType.Exp

    nc = bacc.Bacc("TRN2", target_bir_lowering=False, debug=False, num_devices=8)
    xn = nc.dram_tensor("xn", [BL, N, C], bf16, kind="ExternalInput").ap()
    xt = nc.dram_tensor("xt", [BL, C, N], bf16, kind="ExternalInput").ap()
    wq = nc.dram_tensor("wq", [C, C], bf16, kind="ExternalInput").ap()   # Wq^T
    wk = nc.dram_tensor("wk", [C, C], bf16, kind="ExternalInput").ap()   # Wk natural
    wv = nc.dram_tensor("wv", [C, C], bf16, kind="ExternalInput").ap()   # Wv^T
    wp = nc.dram_tensor("wp", [C, C], bf16, kind="ExternalInput").ap()   # Wproj^T
    bp = nc.dram_tensor("bp", [1, C], f32, kind="ExternalInput").ap()
    xq = nc.dram_tensor("xq", [128, CK * 128], bf16, kind="ExternalInput").ap()
    y = nc.dram_tensor("y", [BL, QL, C], f32, kind="ExternalOutput").ap()

    with tile.TileContext(nc) as tc:
        with (
            tc.tile_pool(name="wpool", bufs=2) as wpool,
            tc.tile_pool(name="xtp", bufs=2) as xtp,
            tc.tile_pool(name="xnp", bufs=3) as xnp,
            tc.tile_pool(name="gpool", bufs=1) as gpool,
            tc.tile_pool(name="upool", bufs=1) as upool,
            tc.tile_pool(name="small", bufs=1) as small,
            tc.tile_pool(name="ptp", bufs=3) as ptp,
            tc.tile_pool(name="ps8", bufs=8, space="PSUM") as ps8,
        ):
            ones32 = small.tile([128, 8], bf16, tag="ones32")
            nc.gpsimd.memset(ones32[:], 1.0)
            bps = small.tile([128, C], bf16, tag="bps")
            nc.gpsimd.dma_start(bps[0:1, :], bp[:, :])
            bpf = small.tile([128, C], bf16, tag="bpf")
            nc.gpsimd.partition_broadcast(bpf[:], bps[0:1, :])

            # xq pre-packed on host: [128, ck*128 + b*64 + q] — one clean DMA
            xqt = small.tile([128, CK, 128], bf16, tag="xqt", name="xqt")
            nc.scalar.dma_start(xqt[:, :, :], xq[:, :])

            # prologue weights, tiles interleaved so q's ck-loop starts early
            wt = wpool.tile([128, 8 * 1024], bf16, tag="w", name="wt_q")
            wt2 = wpool.tile([128, 8 * 1024], bf16, tag="w", name="wt_k")
            for ck in range(CK):
                nc.sync.dma_start(wt[:, ck * 1024:(ck + 1) * 1024],
                                  wq[ck * 128:(ck + 1) * 128, :])
                nc.sync.dma_start(wt2[:, ck * 1024:(ck + 1) * 1024],
                                  wk[ck * 128:(ck + 1) * 128, :])

            # ---------- q^T for both batches ----------
            # psum jc: [hd-of-pair(128), (b0 64q | b1 64q)]; lands in the
            # block-diagonal layout qbd[:, pair, (b 128: h0 64 | h1 64)]
            qbd = small.tile([128, 8, 256], bf16, tag="qbd", name="qbd")
            nc.gpsimd.memset(qbd[:], 0.0)
            gt_all = gpool.tile([128, BL, CK * 1024], bf16, tag="gt",
                                name="gt_all")

            def emit_q(jc):
                ps = ps8.tile([128, 512], f32, tag="ps")
                for ck in range(CK):
                    nc.tensor.matmul(
                        ps[:, 0:128],
                        wt[:, ck * 1024 + jc * 128: ck * 1024 + (jc + 1) * 128],
                        xqt[:, ck, :],
                        start=(ck == 0), stop=(ck == CK - 1))
                for b in range(BL):
                    for sub in range(2):
                        dst = qbd[sub * 64:(sub + 1) * 64, jc,
                                  b * 128 + sub * 64: b * 128 + (sub + 1) * 64]
                        srcp = ps[sub * 64:(sub + 1) * 64, b * 64:(b + 1) * 64]
                        if (b + sub) % 2 == 0:
                            nc.vector.tensor_copy(dst, srcp)
                        else:
                            nc.scalar.copy(dst, srcp)

            def emit_G(ph):
                for ck in range(CK):
                    ps = ps8.tile([128, 512], f32, tag="ps")
                    for k in range(2):
                        pair = ph * 2 + k
                        nc.tensor.matmul(
                            ps[:, k * 256:(k + 1) * 256],
                            wt2[:, pair * 1024 + ck * 128:
                                pair * 1024 + (ck + 1) * 128],
                            qbd[:, pair, :], start=True, stop=True)
                    for k in range(2):
                        pair = ph * 2 + k
                        # one strided copy evicts both batches' pair block
                        dst = gt_all[:, :, ck * 1024 + pair * 128:
                                     ck * 1024 + (pair + 1) * 128]
                        srcp = ps[:, k * 256:(k + 1) * 256].rearrange(
                            "p (b c) -> p b c", c=128)
                        if k == 0:
                            nc.vector.tensor_copy(dst, srcp)
                        else:
                            nc.scalar.copy(dst, srcp)

            # interleave q pairs with their G phase so G fills the PE idle
            # while q paces on the wq DMA stream (the scheduler does not
            # hoist G across the whole q phase on its own)
            for ph in range(4):
                emit_q(2 * ph)
                emit_q(2 * ph + 1)
                emit_G(ph)

            # Pre-warm the x-tile pool slots with a stripe write that
            # RAW-waits on q's final qbd eviction: the real x DMAs then
            # WAW-wait on the stripe, so the whole prologue weight stream
            # gets the HBM bandwidth to itself.  xtt0 stays ungated (it rides
            # the SP queue FIFO behind the weights and S(0) needs it first).
            for pool, nb, width, step in ((xtp, 1, CK * TB, TB),
                                          (xnp, 3, TPB * 1024, 1024)):
                for k in range(nb):
                    dmy = pool.tile([128, width], bf16,
                                    tag=("xt" if pool is xtp else "xn"),
                                    name=f"gate_{pool.name}_{k}")
                    nslc = width // step
                    gv = dmy[0:1, :].rearrange("p (a b) -> p a b", b=step)[:, :, 0:8]
                    srcw = qbd[0:1, 7, 0:nslc * 8].rearrange(
                        "p (a b) -> p a b", b=8)
                    nc.gpsimd.tensor_copy(gv, srcw)


            # u accumulator (bf16, both batches)
            uacc = upool.tile([128, BL, CK * 1024], bf16, name="uacc")
            paccs = []
            for b in range(BL):
                pa = small.tile([128, HQ], f32, tag="pacc", bufs=2,
                                name=f"pacc{b}")
                nc.gpsimd.memset(pa[:], 0.0)
                paccs.append(pa)

            ptcs = {}
            xnts = {}
            rds = {}

            def emit_block(vb):
                b, blk = divmod(vb, NBLK)
                xtt = xtp.tile([128, CK * TB], bf16, tag="xt", name=f"xt{vb}")
                for ck in range(CK):
                    nc.sync.dma_start(
                        xtt[:, ck * TB:(ck + 1) * TB],
                        xt[b, ck * 128:(ck + 1) * 128, blk * TB:(blk + 1) * TB])
                xnt = xnp.tile([128, TPB * 1024], bf16, tag="xn", name=f"xn{vb}")
                for i in range(TPB):
                    nc.scalar.dma_start(
                        xnt[:, i * 1024:(i + 1) * 1024],
                        xn[b, (blk * TPB + i) * 128:(blk * TPB + i + 1) * 128, :])
                xnts[vb] = xnt
                ptc = ptp.tile([128, TPB * 1024], bf16, tag="ptc", name=f"ptc{vb}")
                for i in range(TPB):
                    for qh in range(2):
                        st = ps8.tile([128, 512], f32, tag="ps")
                        for ck in range(CK):
                            nc.tensor.matmul(
                                st[:],
                                xtt[:, ck * TB + i * 128: ck * TB + (i + 1) * 128],
                                gt_all[:, b, ck * 1024 + qh * 512:
                                       ck * 1024 + (qh + 1) * 512],
                                start=(ck == 0), stop=(ck == CK - 1))
                        pslice = ptc[:, i * 1024 + qh * 512: i * 1024 + (qh + 1) * 512]
                        nc.scalar.activation(pslice, st[:], EXP, scale=SCALE)
                        pa = paccs[b][:, qh * 512:(qh + 1) * 512]
                        nc.gpsimd.tensor_add(pa, pslice, pa)
                ptcs[vb] = ptc

            def emit_uround(p, hooks=None):
                b, r = divmod(p, 4)
                v0, v1 = 2 * p, 2 * p + 1
                for qh in range(2):
                    for cq in range(2):
                        if hooks and (st_idx := qh * 2 + cq) in hooks:
                            hooks[st_idx]()
                        ups = [ps8.tile([128, 512], f32, tag="ps",
                                        name=f"ups{p}_{qh}_{cq}_{j}")
                               for j in range(4)]
                        for half, v in enumerate((v0, v1)):
                            pp, xx = ptcs[v], xnts[v]
                            for i in range(TPB):
                                for k4 in range(4):
                                    ck = cq * 4 + k4
                                    nc.tensor.matmul(
                                        ups[k4][:],
                                        xx[:, i * 1024 + ck * 128: i * 1024 + (ck + 1) * 128],
                                        pp[:, i * 1024 + qh * 512: i * 1024 + (qh + 1) * 512],
                                        start=(half == 0 and i == 0),
                                        stop=(half == 1 and i == TPB - 1))
                        for k4 in range(4):
                            ck = cq * 4 + k4
                            dst = uacc[:, b, ck * 1024 + qh * 512: ck * 1024 + (qh + 1) * 512]
                            if r == 0:
                                nc.vector.tensor_copy(dst, ups[k4][:])
                            else:
                                nc.vector.tensor_add(dst, ups[k4][:], dst)
                if hooks and 4 in hooks:
                    hooks[4]()
                del ptcs[v0], ptcs[v1], xnts[v0], xnts[v1]

            def emit_den(b):
                # bf16 copy of pacc: the ones-matmul then runs at 1 cyc/row
                # instead of fp32's 4 (den err ~0.04%: the 128 partials'
                # quantization noise averages out in the fp32 PSUM sum)
                pbf = small.tile([128, HQ], bf16, tag="pbf", name=f"pbf{b}")
                nc.scalar.copy(pbf[:], paccs[b][:])
                rd = small.tile([128, HQ], f32, tag="rd", name=f"rd{b}")
                for qh in range(2):
                    dnp = ps8.tile([128, 512], f32, tag="ps", name=f"dnp{b}_{qh}")
                    nc.tensor.matmul(dnp[0:8, :], ones32[:],
                                     pbf[:, qh * 512:(qh + 1) * 512],
                                     start=True, stop=True)
                    nc.vector.reciprocal(rd[0:1, qh * 512:(qh + 1) * 512],
                                         dnp[0:1, :])
                rdf = small.tile([128, HQ], f32, tag="rdf", name=f"rdf{b}")
                nc.gpsimd.partition_broadcast(rdf[:], rd[0:1, :])
                rds[b] = rdf

            # per-head-pair reciprocal layout for the o^T scale:
            # rdo[p, jc, b*64+qq] = 1/d_b[(2jc + p//64)*64 + qq]
            rdo = small.tile([128, 8, 128], f32, tag="rdo", name="rdo")

            def emit_rdo(b):
                rdf = rds[b]
                for jc in range(8):
                    nc.scalar.copy(
                        rdo[0:64, jc, b * 64:(b + 1) * 64],
                        rdf[0:64, (2 * jc) * 64:(2 * jc + 1) * 64])
                    nc.scalar.copy(
                        rdo[64:128, jc, b * 64:(b + 1) * 64],
                        rdf[64:128, (2 * jc + 1) * 64:(2 * jc + 2) * 64])

            # ---------- flat 16-block pipeline ----------
            wt3 = wt4 = None
            for vb in range(VB):
                emit_block(vb)
                if vb == 1:
                    # epilogue weights ride a slow ring mid-loop
                    wt3 = wpool.tile([128, 8 * 1024], bf16, tag="w", name="wt_v")
                    for ck in range(CK):
                        nc.gpsimd.dma_start(wt3[:, ck * 1024:(ck + 1) * 1024],
                                            wv[ck * 128:(ck + 1) * 128, :])
                if vb == 2:
                    wt4 = wpool.tile([128, 8 * 1024], bf16, tag="w", name="wt_p")
                    for jc in range(CK):
                        nc.gpsimd.dma_start(wt4[:, jc * 1024:(jc + 1) * 1024],
                                            wp[jc * 128:(jc + 1) * 128, :])
                if vb >= 2 and vb % 2 == 0:
                    p = vb // 2 - 1
                    if p == 3:
                        emit_uround(p, hooks={2: lambda: (emit_den(0),
                                                          emit_rdo(0))})
                    else:
                        emit_uround(p)

            # ---------- joint epilogue ----------
            # o^T per head pair, moving spans both batches via strided AP.
            # jc 0-3 read only qh0 columns of uacc, so they interleave into
            # the back half of the last u-round.
            oT = small.tile([128, 8, 128], bf16, tag="oT", name="oT")

            def emit_o(jcs):
                for jc in jcs:
                    ps = ps8.tile([128, 512], f32, tag="ps")
                    for sub in range(2):
                        h = jc * 2 + sub
                        ucol = (h // 8) * 512 + (h % 8) * 64
                        for ck in range(CK):
                            nc.tensor.matmul(
                                ps[sub * 64:(sub + 1) * 64, 0:128],
                                wt3[:, ck * 1024 + h * 64: ck * 1024 + (h + 1) * 64],
                                uacc[:, :, ck * 1024 + ucol: ck * 1024 + ucol + 64],
                                start=(ck == 0), stop=(ck == CK - 1),
                                tile_position=(0, sub * 64))
                    nc.vector.tensor_mul(oT[:, jc, :], ps[:, 0:128], rdo[:, jc, :])

            # last u-round with b1's denominator chain and the qh0-half of o
            # slotted into its set boundaries
            emit_uround(7, hooks={
                2: lambda: (emit_den(1), emit_rdo(1)),
                3: lambda: emit_o((0, 1)),
                4: lambda: emit_o((2, 3)),
            })
            emit_o((4, 5, 6, 7))

            for b in range(BL):
                ys = small.tile([128, C], f32, tag="ys", bufs=2, name=f"ys{b}")
                for half in range(2):
                    ps = ps8.tile([128, 512], f32, tag="ps")
                    for jc in range(8):
                        nc.tensor.matmul(
                            ps[0:QL, :],
                            oT[:, jc, b * 64:(b + 1) * 64],
                            wt4[:, jc * 1024 + half * 512: jc * 1024 + (half + 1) * 512],
                            start=(jc == 0), stop=(jc == 7))
                    nc.vector.tensor_add(
                        ys[0:QL, half * 512:(half + 1) * 512], ps[0:QL, :],
                        bpf[0:QL, half * 512:(half + 1) * 512])
                    nc.sync.dma_start(y[b, :, half * 512:(half + 1) * 512],
                                      ys[0:QL, half * 512:(half + 1) * 512])

    nc.compile()
    return nc


def get_nc():
    if "nc" not in _CACHE:
        _CACHE["nc"] = _build()
    return _CACHE["nc"]


def make_in_maps(x, Wq, Wk, Wv, Wproj, bproj):
    import ml_dtypes
    bf = ml_dtypes.bfloat16
    x = np.ascontiguousarray(x, dtype=np.float32)
    xt32 = np.ascontiguousarray(x.transpose(0, 2, 1))
    # xq packed per core: [128, ck*128 + b*64 + q]
    xqb = xt32[:, :, 0:QL].reshape(B, CK, 128, QL).astype(bf)
    xtb = xt32.astype(bf)
    xnb = x.astype(bf)
    wqb = np.ascontiguousarray(np.asarray(Wq, dtype=np.float32).T).astype(bf)
    wkb = np.ascontiguousarray(np.asarray(Wk, dtype=np.float32)).astype(bf)
    wvb = np.ascontiguousarray(np.asarray(Wv, dtype=np.float32).T).astype(bf)
    wpb = np.ascontiguousarray(np.asarray(Wproj, dtype=np.float32).T).astype(bf)
    bpf = np.ascontiguousarray(np.asarray(bproj, dtype=np.float32).reshape(1, C))
    in_maps = []
    for core in range(8):
        s = slice(core * BL, (core + 1) * BL)
        # [BL, CK, 128, QL] -> [128, CK, BL, QL] -> [128, CK*BL*QL]
        xqp = np.ascontiguousarray(
            xqb[s].transpose(2, 1, 0, 3).reshape(128, CK * BL * QL))
        in_maps.append({
            "xn": np.ascontiguousarray(xnb[s]),
            "xt": np.ascontiguousarray(xtb[s]),
            "xq": xqp,
            "wq": wqb, "wk": wkb, "wv": wvb, "wp": wpb, "bp": bpf,
        })
    return in_maps


def kernel(x, Wq, Wk, Wv, Wproj, bproj):
    from concourse import bass_utils
    nc = get_nc()
    in_maps = make_in_maps(x, Wq, Wk, Wv, Wproj, bproj)
    res = bass_utils.run_bass_kernel_spmd(nc, in_maps, core_ids=list(range(8)))
    out = np.concatenate([res.results[i]["y"] for i in range(8)], axis=0)
    return out.astype(np.float32)
